# revision 7
# baseline (speedup 1.0000x reference)
"""Trainium2 Bass kernel for AttentionBilinear.

Per batch b:
    pW     = p[b] @ W                         # [Tp, Dq]
    scores = pW @ q[b].T                      # [Tp, Tq]
    wts    = softmax(scores, axis=Tp)
    out[b] = wts @ q[b]                       # [Tp, Dq]

Computed in the transposed-scores orientation so the softmax over Tp is a
free-axis reduction:
    pWT[d, tp]  = sum_e W[e, d] * pT[e, tp]       (mm1: lhsT=W,  rhs=pT)
    scT[tq, tp] = sum_d qT[d, tq] * pWT[d, tp]    (mm2: lhsT=qT, rhs=pWT)
    softmax over tp (free axis), read from PSUM   (DVE max / ACT exp / DVE mul)
    out[tp, d]  = sum_tq wT[tq, tp] * q[tq, d]    (mm3: lhsT=wT, rhs=q)

All matmuls run in fp16 (1 cycle/row on TRN2; fp8 DoubleRow measures the
same 215ns/instr for 2x the contraction, but single-term fp8 fails the
2e-2 gate at 3.5e-2, and multi-term splits erase the speed win — so fp16
is the PE floor: 768 matmuls x 216ns = 166us/core).

The schedule targets zero PE stalls after the framework prologue (~6us,
fixed): W is host-blocked k-major so one contiguous 256KB DMA delivers
the stationary tiles for a whole contraction chunk, batch 0's first pT
half arrives as 8 fine-grained 128KB pieces, and mm1(b=0, n=0) runs
k-outer across 8 PSUM banks so the PE starts on chunk 0 while chunk 1 is
still in flight. Output is staged to fp16 (host upcasts) and stored on
the scalar ring, which is idle after the W loads.

Sharding: data-parallel over batch B=16 across 8 cores, W replicated.
"""

import numpy as np

P = 128   # partitions
H = 512   # PSUM bank width in fp32

B_FULL = 16
T_FULL = 1024
D_FULL = 1024
N_CORES = 8

MODE = ("hi", "hi")  # kept for test.py's banner


def build_nc(b_loc=2, t=1024, d=1024):
    from contextlib import ExitStack

    import concourse.tile as tile
    from concourse import bacc, mybir

    f32 = mybir.dt.float32
    f16 = mybir.dt.float16
    C = t // P     # row chunks of a [t, d] matrix
    KC = d // P    # chunks of the d (feature) axis
    TH = t // H    # 512-wide pieces of the t axis
    NH = d // H    # 512-wide pieces of the d axis
    AX = mybir.AxisListType.X
    EXP = mybir.ActivationFunctionType.Exp
    MIN = mybir.AluOpType.min
    ADD = mybir.AluOpType.add

    nc = bacc.Bacc()

    # Inputs are host-packed partition-major ([b, p, c, cols]) so every DMA
    # descriptor is an 8-16KB contiguous run (vs 1-2KB row-major).
    def dram_in(name):
        return nc.dram_tensor(
            name, [b_loc, P, C, d], f16, kind="ExternalInput"
        ).ap()

    qh_ext = dram_in("qh")          # q natural, fp16
    qt_ext = dram_in("qt")          # q transposed per batch: [d, tq]
    # p transposed, additionally split into tp-halves: [b, TH, p, c, H]
    pt_ext = nc.dram_tensor(
        "pt", [b_loc, TH, P, C, H], f16, kind="ExternalInput"
    ).ap()
    # W host-blocked k-major as [k, p, m, c] = W[k*128+p, m*128+c]: the
    # piece for contraction chunk k is one contiguous 256KB DMA carrying
    # the stationary tiles for ALL output chunks m.
    w_ext = nc.dram_tensor("w", [KC, P, KC, P], f16, kind="ExternalInput").ap()
    # Output staged fp16 (host upcasts to fp32): halves the store traffic.
    out_ext = nc.dram_tensor("out", [b_loc, t, d], f16, kind="ExternalOutput").ap()

    with tile.TileContext(nc) as tc, ExitStack() as ctx:
        consts = ctx.enter_context(tc.tile_pool(name="consts", bufs=1))
        qh_pool = ctx.enter_context(tc.tile_pool(name="qh_pool", bufs=2))
        qt_pool = ctx.enter_context(tc.tile_pool(name="qt_pool", bufs=2))
        pt_pool = ctx.enter_context(tc.tile_pool(name="pt_pool", bufs=2))
        pwt_pool = ctx.enter_context(tc.tile_pool(name="pwt_pool", bufs=2))
        wt_pool = ctx.enter_context(tc.tile_pool(name="wt_pool", bufs=2))
        ostage = ctx.enter_context(tc.tile_pool(name="ostage", bufs=4))
        stats = ctx.enter_context(tc.tile_pool(name="stats", bufs=2))
        psum_mm = ctx.enter_context(tc.tile_pool(name="psum_mm", bufs=8, space="PSUM"))

        # ---- PE ramp warm-up: the clock takes ~3us of continuous busy to
        # reach 2.4GHz, and the first real data can't land before ~9.5us
        # (prologue + DMA pipeline). Nonzero junk matmuls burn the ramp
        # during the DMA wait so every real matmul runs at full clock.
        # (memset-zero operands get zero-skipped in ~40ns and don't ramp.)
        warm = consts.tile([P, H], f16, name="warm")
        nc.gpsimd.memset(warm[:], 1.0)
        wacc = psum_mm.tile([P, H], f32, name="wacc", tag="acc")
        for i in range(6):
            nc.tensor.matmul(
                wacc[:], warm[:, 0:P], warm[:], start=(i == 0), stop=(i == 5)
            )

        # ---- W resident (fp16), k-major blocks; lhsT for (k, m) is
        # w_[:, k, m, :]. One contiguous 256KB DMA per k, interleaved with
        # batch 0's pT pieces across BOTH rings in consumption order (even
        # chunks on sync, odd on scalar) so mm1's k-outer startup survives
        # the chip-wide HBM burst while all 8 cores load at once. ----
        w_ = consts.tile([P, KC, KC, P], f16, name="w_hi")

        st = [dict() for _ in range(b_loc)]

        def load_mat(pool, name, tag, ext, b, engine):
            """packed [b, P, C, d] DRAM (fp16) -> [P, C, d] SBUF in one DMA."""
            mt = pool.tile([P, C, d], f16, name=name, tag=tag)
            engine.dma_start(mt[:], ext[b])
            return mt

        def phase_loads(b):
            # All loads ride the sync ring as one FIFO in exact consumption
            # order, so early phases are never starved by later tensors.
            if b == 0:
                # Batch 0 startup: (W chunk k, pT half-0 chunk k) pairs in
                # exact k-outer consumption order, striped across both DMA
                # rings — each ring only needs ~110GB/s to keep the PE fed.
                pt0 = pt_pool.tile([P, C, d], f16, name="pT_0", tag="pT")
                for k in range(KC):
                    eng = nc.sync if k % 2 == 0 else nc.scalar
                    if k == 0:
                        # First chunk lands in m-sized bites so matmul
                        # (k=0, m=0) fires after ~160KB, not ~384KB.
                        nc.sync.dma_start(w_[:, 0, 0], w_ext[0, :, 0])
                        nc.sync.dma_start(pt0[:, 0, 0:H], pt_ext[0, 0, :, 0])
                        for m in range(1, KC):
                            nc.sync.dma_start(w_[:, 0, m], w_ext[0, :, m])
                        continue
                    eng.dma_start(w_[:, k], w_ext[k])
                    eng.dma_start(pt0[:, k, 0:H], pt_ext[0, 0, :, k])
                for h in range(1, TH):
                    nc.sync.dma_start(
                        pt0[:, :, h * H : (h + 1) * H], pt_ext[0, h]
                    )
                st[0]["pT"] = pt0
            else:
                mt = pt_pool.tile([P, C, d], f16, name=f"pT_{b}", tag="pT")
                for h in range(TH):
                    nc.sync.dma_start(mt[:, :, h * H : (h + 1) * H], pt_ext[b, h])
                st[b]["pT"] = mt
            st[b]["qT"] = load_mat(qt_pool, f"qT_{b}", "qT", qt_ext, b, nc.sync)
            st[b]["qh"] = load_mat(qh_pool, f"qh_{b}", "qh", qh_ext, b, nc.sync)

        def phase_mm1(b):
            """pWT[d, tp] = sum_e W[e,d] * pT[e,tp]."""
            pT = st[b]["pT"]
            pWT = pwt_pool.tile([P, KC, t], f16, name=f"pWT_{b}", tag="pWT")
            if b == 0:
                # n=0 k-outer across 8 banks: each arriving (pT chunk k,
                # W chunk k) pair feeds 8 matmuls, so the PE is paced by
                # compute (1.7us/chunk) not DMA (0.45us/chunk) from the
                # first piece on.
                accs = [
                    psum_mm.tile([P, H], f32, name=f"a1_0_{m}_0", tag="acc")
                    for m in range(KC)
                ]
                for k in range(KC):
                    for m in range(KC):
                        nc.tensor.matmul(
                            accs[m][:],
                            w_[:, k, m, :],
                            pT[:, k, 0:H],
                            start=(k == 0),
                            stop=(k == KC - 1),
                        )
                for m in range(KC):
                    nc.scalar.copy(pWT[:, m, 0:H], accs[m][:])
                n_range = range(1, TH)
            else:
                n_range = range(TH)
            for n in n_range:
                n_sl = slice(n * H, (n + 1) * H)
                for m in range(KC):
                    acc = psum_mm.tile([P, H], f32, name=f"a1_{b}_{m}_{n}", tag="acc")
                    for k in range(KC):
                        nc.tensor.matmul(
                            acc[:],
                            w_[:, k, m, :],
                            pT[:, k, n_sl],
                            start=(k == 0),
                            stop=(k == KC - 1),
                        )
                    nc.scalar.copy(pWT[:, m, n_sl], acc[:])
            st[b]["pWT"] = pWT

        def phase_mm2sm(b):
            """scores into PSUM; softmax straight out of PSUM into fp16 wT."""
            qT = st[b]["qT"]
            pWT = st[b]["pWT"]
            wT = wt_pool.tile([P, C, t], f16, name=f"wT_{b}", tag="wT")
            negmax = stats.tile([P, C, TH], f32, name=f"negmax_{b}", tag="negmax")
            nm = stats.tile([P, C], f32, name=f"nm_{b}", tag="nm")
            sume = stats.tile([P, C, TH], f32, name=f"sume_{b}", tag="sume")
            recip = stats.tile([P, C], f32, name=f"recip_{b}", tag="recip")
            for m in range(C):
                msl = slice(m * P, (m + 1) * P)
                accs = []
                for n in range(TH):
                    n_sl = slice(n * H, (n + 1) * H)
                    acc = psum_mm.tile([P, H], f32, name=f"a2_{b}_{m}_{n}", tag="acc")
                    for k in range(KC):
                        nc.tensor.matmul(
                            acc[:],
                            qT[:, k, msl],
                            pWT[:, k, n_sl],
                            start=(k == 0),
                            stop=(k == KC - 1),
                        )
                    nc.vector.reduce_max(
                        negmax[:, m, n : n + 1], acc[:], axis=AX, negate=True
                    )
                    accs.append(acc)
                if TH > 1:
                    nc.vector.tensor_tensor(
                        nm[:, m : m + 1], negmax[:, m, 0:1], negmax[:, m, 1:2], op=MIN
                    )
                    nm_sl = nm[:, m : m + 1]
                else:
                    nm_sl = negmax[:, m, 0:1]
                for n, acc in enumerate(accs):
                    nc.scalar.activation(
                        wT[:, m, n * H : (n + 1) * H],
                        acc[:],
                        EXP,
                        bias=nm_sl,
                        accum_out=sume[:, m, n : n + 1],
                    )
                if TH > 1:
                    nc.vector.tensor_tensor(
                        recip[:, m : m + 1], sume[:, m, 0:1], sume[:, m, 1:2], op=ADD
                    )
                    nc.vector.reciprocal(recip[:, m : m + 1], recip[:, m : m + 1])
                else:
                    nc.vector.reciprocal(recip[:, m : m + 1], sume[:, m, 0:1])
                nc.vector.tensor_scalar_mul(wT[:, m, :], wT[:, m, :], recip[:, m : m + 1])
            st[b]["wT"] = wT

        def phase_mm3(b):
            """out[tp, d] = sum_tq wT[tq,tp] * qh[tq,d]."""
            wT = st[b]["wT"]
            qh = st[b]["qh"]
            for m in range(C):
                msl = slice(m * P, (m + 1) * P)
                for n in range(NH):
                    n_sl = slice(n * H, (n + 1) * H)
                    acc = psum_mm.tile([P, H], f32, name=f"a3_{b}_{m}_{n}", tag="acc")
                    for k in range(KC):
                        nc.tensor.matmul(
                            acc[:],
                            wT[:, k, msl],
                            qh[:, k, n_sl],
                            start=(k == 0),
                            stop=(k == KC - 1),
                        )
                    ot = ostage.tile([P, H], f16, name=f"ot_{b}_{m}_{n}", tag="ot")
                    last = b == b_loc - 1 and m == C - 1 and n == NH - 1
                    # The very last tile drains in two pipelined halves so
                    # the copy of half 2 overlaps the DMA of half 1.
                    n_pieces = 2 if last else 1
                    W2 = H // n_pieces
                    for i in range(n_pieces):
                        isl = slice(i * W2, (i + 1) * W2)
                        nc.scalar.copy(ot[:, isl], acc[:, isl])
                        nc.scalar.dma_start(
                            out_ext[
                                b,
                                m * P : (m + 1) * P,
                                n * H + i * W2 : n * H + (i + 1) * W2,
                            ],
                            ot[:, isl],
                        )

        # Emission order = per-engine program order. Batch b+1's mm1 is
        # emitted before batch b's mm3 so the PE stays busy while b's softmax
        # tail completes.
        phase_loads(0)
        phase_mm1(0)
        for b in range(b_loc):
            phase_mm2sm(b)
            if b + 1 < b_loc:
                phase_loads(b + 1)
                phase_mm1(b + 1)
            phase_mm3(b)

    nc.finalize()  # run the Bacc legalization/regalloc passes for walrus
    return nc


_CACHE = {}


def _get_nc():
    if "nc" not in _CACHE:
        _CACHE["nc"] = build_nc(B_FULL // N_CORES, T_FULL, D_FULL)
    return _CACHE["nc"]


def _prep_inputs(q, p, W):
    """Host-side layout prep: fp16 casts and per-batch transposes."""
    q = np.ascontiguousarray(q, dtype=np.float32)
    p = np.ascontiguousarray(p, dtype=np.float32)
    W = np.ascontiguousarray(W, dtype=np.float32)
    d = W.shape[0]
    KC = d // P
    t = q.shape[1]
    C = t // P
    TH = t // H

    def pack(x16):
        # [b, t, cols] -> [b, p, c, cols]: 16KB contiguous per partition
        b, _, cols = x16.shape
        return np.ascontiguousarray(
            x16.reshape(b, C, P, cols).transpose(0, 2, 1, 3)
        )

    def pack_halved(x16):
        # [b, t, cols] -> [b, h, p, c, H]: 8KB contiguous per partition
        b, _, cols = x16.shape
        return np.ascontiguousarray(
            x16.reshape(b, C, P, TH, H).transpose(0, 3, 2, 1, 4)
        )

    qh = q.astype(np.float16)
    qt = np.transpose(qh, (0, 2, 1))
    pt = np.transpose(p, (0, 2, 1)).astype(np.float16)
    # k-major W blocks: [k, p, m, c] = W[k*128+p, m*128+c] — a plain reshape
    wh = np.ascontiguousarray(W.astype(np.float16).reshape(KC, P, KC, P))
    return {
        "qh": pack(qh),
        "qt": pack(qt),
        "pt": pack_halved(pt),
        "w": wh,
    }


def run(q, p, W, nc=None, **spmd_kwargs):
    """Run on 8 NeuronCores; returns (out, BassKernelResults)."""
    from concourse.bass_utils import run_bass_kernel_spmd

    arrs = _prep_inputs(q, p, W)
    if nc is None:
        nc = _get_nc()
    bl = B_FULL // N_CORES
    batch_sharded = {"qh", "qt", "pt"}
    in_maps = []
    for i in range(N_CORES):
        m = {}
        for name, a in arrs.items():
            m[name] = a[i * bl : (i + 1) * bl] if name in batch_sharded else a
        in_maps.append(m)
    res = run_bass_kernel_spmd(nc, in_maps, list(range(N_CORES)), **spmd_kwargs)
    out = np.concatenate(
        [res.results[i]["out"].astype(np.float32) for i in range(N_CORES)], axis=0
    )
    return out, res


def kernel(q, p, W):
    out, _ = run(q, p, W)
    return out


# revision 9
# speedup vs baseline: 1.0171x; 1.0171x over previous
"""Trainium2 Bass kernel for AttentionBilinear.

Per batch b:
    pW     = p[b] @ W                         # [Tp, Dq]
    scores = pW @ q[b].T                      # [Tp, Tq]
    wts    = softmax(scores, axis=Tp)
    out[b] = wts @ q[b]                       # [Tp, Dq]

Computed in the transposed-scores orientation so the softmax over Tp is a
free-axis reduction:
    pWT[d, tp]  = sum_e W[e, d] * pT[e, tp]       (mm1: lhsT=W,  rhs=pT)
    scT[tq, tp] = sum_d qT[d, tq] * pWT[d, tp]    (mm2: lhsT=qT, rhs=pWT)
    softmax over tp (free axis), read from PSUM   (DVE max / ACT exp / DVE mul)
    out[tp, d]  = sum_tq wT[tq, tp] * q[tq, d]    (mm3: lhsT=wT, rhs=q)

All matmuls run in fp16 (1 cycle/row on TRN2; fp8 DoubleRow measures the
same 215ns/instr for 2x the contraction, but single-term fp8 fails the
2e-2 gate at 3.5e-2, and multi-term splits erase the speed win — so fp16
is the PE floor: 768 matmuls x 216ns = 166us/core).

The schedule targets zero PE stalls after the framework prologue (~6us,
fixed): W is host-blocked k-major so one contiguous 256KB DMA delivers
the stationary tiles for a whole contraction chunk, batch 0's first pT
half arrives as 8 fine-grained 128KB pieces, and mm1(b=0, n=0) runs
k-outer across 8 PSUM banks so the PE starts on chunk 0 while chunk 1 is
still in flight. Output is staged to fp16 (host upcasts) and stored on
the scalar ring, which is idle after the W loads.

Sharding: data-parallel over batch B=16 across 8 cores, W replicated.
"""

import numpy as np

P = 128   # partitions
H = 512   # PSUM bank width in fp32

B_FULL = 16
T_FULL = 1024
D_FULL = 1024
N_CORES = 8

MODE = ("hi", "hi")  # kept for test.py's banner


def build_nc(b_loc=2, t=1024, d=1024):
    from contextlib import ExitStack

    import concourse.tile as tile
    from concourse import bacc, mybir

    f32 = mybir.dt.float32
    f16 = mybir.dt.float16
    C = t // P     # row chunks of a [t, d] matrix
    KC = d // P    # chunks of the d (feature) axis
    TH = t // H    # 512-wide pieces of the t axis
    NH = d // H    # 512-wide pieces of the d axis
    AX = mybir.AxisListType.X
    EXP = mybir.ActivationFunctionType.Exp
    MIN = mybir.AluOpType.min
    ADD = mybir.AluOpType.add

    nc = bacc.Bacc()

    # Inputs are host-packed partition-major ([b, p, c, cols]) so every DMA
    # descriptor is an 8-16KB contiguous run (vs 1-2KB row-major).
    def dram_in(name):
        return nc.dram_tensor(
            name, [b_loc, P, C, d], f16, kind="ExternalInput"
        ).ap()

    qh_ext = dram_in("qh")          # q natural, fp16
    qt_ext = dram_in("qt")          # q transposed per batch: [d, tq]
    # p transposed, additionally split into tp-halves: [b, TH, p, c, H]
    pt_ext = nc.dram_tensor(
        "pt", [b_loc, TH, P, C, H], f16, kind="ExternalInput"
    ).ap()
    # W host-blocked k-major as [k, p, m, c] = W[k*128+p, m*128+c]: the
    # piece for contraction chunk k is one contiguous 256KB DMA carrying
    # the stationary tiles for ALL output chunks m.
    w_ext = nc.dram_tensor("w", [KC, P, KC, P], f16, kind="ExternalInput").ap()
    # Output staged fp16 (host upcasts to fp32): halves the store traffic.
    out_ext = nc.dram_tensor("out", [b_loc, t, d], f16, kind="ExternalOutput").ap()

    with tile.TileContext(nc) as tc, ExitStack() as ctx:
        consts = ctx.enter_context(tc.tile_pool(name="consts", bufs=1))
        qh_pool = ctx.enter_context(tc.tile_pool(name="qh_pool", bufs=2))
        qt_pool = ctx.enter_context(tc.tile_pool(name="qt_pool", bufs=2))
        pt_pool = ctx.enter_context(tc.tile_pool(name="pt_pool", bufs=2))
        pwt_pool = ctx.enter_context(tc.tile_pool(name="pwt_pool", bufs=2))
        wt_pool = ctx.enter_context(tc.tile_pool(name="wt_pool", bufs=2))
        ostage = ctx.enter_context(tc.tile_pool(name="ostage", bufs=4))
        stats = ctx.enter_context(tc.tile_pool(name="stats", bufs=2))
        psum_mm = ctx.enter_context(tc.tile_pool(name="psum_mm", bufs=8, space="PSUM"))

        # ---- PE ramp warm-up: the clock takes ~3us of continuous busy to
        # reach 2.4GHz, and the first real data can't land before ~9.5us
        # (prologue + DMA pipeline). Nonzero junk matmuls burn the ramp
        # during the DMA wait so every real matmul runs at full clock.
        # (memset-zero operands get zero-skipped in ~40ns and don't ramp.)
        warm = consts.tile([P, H], f16, name="warm")
        nc.gpsimd.memset(warm[:], 1.0)
        wacc = psum_mm.tile([P, H], f32, name="wacc", tag="acc")
        for i in range(8):
            nc.tensor.matmul(
                wacc[:], warm[:, 0:P], warm[:], start=(i == 0), stop=(i == 7)
            )

        # ---- W resident (fp16), k-major blocks; lhsT for (k, m) is
        # w_[:, k, m, :]. One contiguous 256KB DMA per k, interleaved with
        # batch 0's pT pieces across BOTH rings in consumption order (even
        # chunks on sync, odd on scalar) so mm1's k-outer startup survives
        # the chip-wide HBM burst while all 8 cores load at once. ----
        w_ = consts.tile([P, KC, KC, P], f16, name="w_hi")

        st = [dict() for _ in range(b_loc)]

        def load_mat(pool, name, tag, ext, b, engine):
            """packed [b, P, C, d] DRAM (fp16) -> [P, C, d] SBUF in one DMA."""
            mt = pool.tile([P, C, d], f16, name=name, tag=tag)
            engine.dma_start(mt[:], ext[b])
            return mt

        def phase_loads(b):
            # All loads ride the sync ring as one FIFO in exact consumption
            # order, so early phases are never starved by later tensors.
            if b == 0:
                # Batch 0 startup: (W chunk k, pT half-0 chunk k) pairs in
                # exact k-outer consumption order, striped across both DMA
                # rings — each ring only needs ~110GB/s to keep the PE fed.
                pt0 = pt_pool.tile([P, C, d], f16, name="pT_0", tag="pT")
                for k in range(KC):
                    eng = nc.sync if k % 2 == 0 else nc.scalar
                    eng.dma_start(w_[:, k], w_ext[k])
                    eng.dma_start(pt0[:, k, 0:H], pt_ext[0, 0, :, k])
                for h in range(1, TH):
                    nc.sync.dma_start(
                        pt0[:, :, h * H : (h + 1) * H], pt_ext[0, h]
                    )
                st[0]["pT"] = pt0
            else:
                mt = pt_pool.tile([P, C, d], f16, name=f"pT_{b}", tag="pT")
                for h in range(TH):
                    nc.sync.dma_start(mt[:, :, h * H : (h + 1) * H], pt_ext[b, h])
                st[b]["pT"] = mt
            st[b]["qT"] = load_mat(qt_pool, f"qT_{b}", "qT", qt_ext, b, nc.sync)
            st[b]["qh"] = load_mat(qh_pool, f"qh_{b}", "qh", qh_ext, b, nc.sync)

        def phase_mm1(b):
            """pWT[d, tp] = sum_e W[e,d] * pT[e,tp]."""
            pT = st[b]["pT"]
            pWT = pwt_pool.tile([P, KC, t], f16, name=f"pWT_{b}", tag="pWT")
            if b == 0:
                # n=0 k-outer across 8 banks: each arriving (pT chunk k,
                # W chunk k) pair feeds 8 matmuls, so the PE is paced by
                # compute (1.7us/chunk) not DMA (0.45us/chunk) from the
                # first piece on.
                accs = [
                    psum_mm.tile([P, H], f32, name=f"a1_0_{m}_0", tag="acc")
                    for m in range(KC)
                ]
                for k in range(KC):
                    for m in range(KC):
                        nc.tensor.matmul(
                            accs[m][:],
                            w_[:, k, m, :],
                            pT[:, k, 0:H],
                            start=(k == 0),
                            stop=(k == KC - 1),
                        )
                for m in range(KC):
                    nc.scalar.copy(pWT[:, m, 0:H], accs[m][:])
                n_range = range(1, TH)
            else:
                n_range = range(TH)
            for n in n_range:
                n_sl = slice(n * H, (n + 1) * H)
                for m in range(KC):
                    acc = psum_mm.tile([P, H], f32, name=f"a1_{b}_{m}_{n}", tag="acc")
                    for k in range(KC):
                        nc.tensor.matmul(
                            acc[:],
                            w_[:, k, m, :],
                            pT[:, k, n_sl],
                            start=(k == 0),
                            stop=(k == KC - 1),
                        )
                    nc.scalar.copy(pWT[:, m, n_sl], acc[:])
            st[b]["pWT"] = pWT

        def phase_mm2sm(b):
            """scores into PSUM; softmax straight out of PSUM into fp16 wT."""
            qT = st[b]["qT"]
            pWT = st[b]["pWT"]
            wT = wt_pool.tile([P, C, t], f16, name=f"wT_{b}", tag="wT")
            negmax = stats.tile([P, C, TH], f32, name=f"negmax_{b}", tag="negmax")
            nm = stats.tile([P, C], f32, name=f"nm_{b}", tag="nm")
            sume = stats.tile([P, C, TH], f32, name=f"sume_{b}", tag="sume")
            recip = stats.tile([P, C], f32, name=f"recip_{b}", tag="recip")
            for m in range(C):
                msl = slice(m * P, (m + 1) * P)
                accs = []
                for n in range(TH):
                    n_sl = slice(n * H, (n + 1) * H)
                    acc = psum_mm.tile([P, H], f32, name=f"a2_{b}_{m}_{n}", tag="acc")
                    for k in range(KC):
                        nc.tensor.matmul(
                            acc[:],
                            qT[:, k, msl],
                            pWT[:, k, n_sl],
                            start=(k == 0),
                            stop=(k == KC - 1),
                        )
                    nc.vector.reduce_max(
                        negmax[:, m, n : n + 1], acc[:], axis=AX, negate=True
                    )
                    accs.append(acc)
                if TH > 1:
                    nc.vector.tensor_tensor(
                        nm[:, m : m + 1], negmax[:, m, 0:1], negmax[:, m, 1:2], op=MIN
                    )
                    nm_sl = nm[:, m : m + 1]
                else:
                    nm_sl = negmax[:, m, 0:1]
                for n, acc in enumerate(accs):
                    nc.scalar.activation(
                        wT[:, m, n * H : (n + 1) * H],
                        acc[:],
                        EXP,
                        bias=nm_sl,
                        accum_out=sume[:, m, n : n + 1],
                    )
                if TH > 1:
                    nc.vector.tensor_tensor(
                        recip[:, m : m + 1], sume[:, m, 0:1], sume[:, m, 1:2], op=ADD
                    )
                    nc.vector.reciprocal(recip[:, m : m + 1], recip[:, m : m + 1])
                else:
                    nc.vector.reciprocal(recip[:, m : m + 1], sume[:, m, 0:1])
                nc.vector.tensor_scalar_mul(wT[:, m, :], wT[:, m, :], recip[:, m : m + 1])
            st[b]["wT"] = wT

        def phase_mm3(b):
            """out[tp, d] = sum_tq wT[tq,tp] * qh[tq,d]."""
            wT = st[b]["wT"]
            qh = st[b]["qh"]
            for m in range(C):
                msl = slice(m * P, (m + 1) * P)
                for n in range(NH):
                    n_sl = slice(n * H, (n + 1) * H)
                    acc = psum_mm.tile([P, H], f32, name=f"a3_{b}_{m}_{n}", tag="acc")
                    for k in range(KC):
                        nc.tensor.matmul(
                            acc[:],
                            wT[:, k, msl],
                            qh[:, k, n_sl],
                            start=(k == 0),
                            stop=(k == KC - 1),
                        )
                    ot = ostage.tile([P, H], f16, name=f"ot_{b}_{m}_{n}", tag="ot")
                    last = b == b_loc - 1 and m == C - 1 and n == NH - 1
                    # The very last tile drains in two pipelined halves so
                    # the copy of half 2 overlaps the DMA of half 1.
                    n_pieces = 2 if last else 1
                    W2 = H // n_pieces
                    for i in range(n_pieces):
                        isl = slice(i * W2, (i + 1) * W2)
                        nc.scalar.copy(ot[:, isl], acc[:, isl])
                        nc.scalar.dma_start(
                            out_ext[
                                b,
                                m * P : (m + 1) * P,
                                n * H + i * W2 : n * H + (i + 1) * W2,
                            ],
                            ot[:, isl],
                        )

        # Emission order = per-engine program order. Batch b+1's mm1 is
        # emitted before batch b's mm3 so the PE stays busy while b's softmax
        # tail completes.
        phase_loads(0)
        phase_mm1(0)
        for b in range(b_loc):
            phase_mm2sm(b)
            if b + 1 < b_loc:
                phase_loads(b + 1)
                phase_mm1(b + 1)
            phase_mm3(b)

    nc.finalize()  # run the Bacc legalization/regalloc passes for walrus
    return nc


_CACHE = {}


def _get_nc():
    if "nc" not in _CACHE:
        _CACHE["nc"] = build_nc(B_FULL // N_CORES, T_FULL, D_FULL)
    return _CACHE["nc"]


def _prep_inputs(q, p, W):
    """Host-side layout prep: fp16 casts and per-batch transposes."""
    q = np.ascontiguousarray(q, dtype=np.float32)
    p = np.ascontiguousarray(p, dtype=np.float32)
    W = np.ascontiguousarray(W, dtype=np.float32)
    d = W.shape[0]
    KC = d // P
    t = q.shape[1]
    C = t // P
    TH = t // H

    def pack(x16):
        # [b, t, cols] -> [b, p, c, cols]: 16KB contiguous per partition
        b, _, cols = x16.shape
        return np.ascontiguousarray(
            x16.reshape(b, C, P, cols).transpose(0, 2, 1, 3)
        )

    def pack_halved(x16):
        # [b, t, cols] -> [b, h, p, c, H]: 8KB contiguous per partition
        b, _, cols = x16.shape
        return np.ascontiguousarray(
            x16.reshape(b, C, P, TH, H).transpose(0, 3, 2, 1, 4)
        )

    qh = q.astype(np.float16)
    qt = np.transpose(qh, (0, 2, 1))
    pt = np.transpose(p, (0, 2, 1)).astype(np.float16)
    # k-major W blocks: [k, p, m, c] = W[k*128+p, m*128+c] — a plain reshape
    wh = np.ascontiguousarray(W.astype(np.float16).reshape(KC, P, KC, P))
    return {
        "qh": pack(qh),
        "qt": pack(qt),
        "pt": pack_halved(pt),
        "w": wh,
    }


def run(q, p, W, nc=None, **spmd_kwargs):
    """Run on 8 NeuronCores; returns (out, BassKernelResults)."""
    from concourse.bass_utils import run_bass_kernel_spmd

    arrs = _prep_inputs(q, p, W)
    if nc is None:
        nc = _get_nc()
    bl = B_FULL // N_CORES
    batch_sharded = {"qh", "qt", "pt"}
    in_maps = []
    for i in range(N_CORES):
        m = {}
        for name, a in arrs.items():
            m[name] = a[i * bl : (i + 1) * bl] if name in batch_sharded else a
        in_maps.append(m)
    res = run_bass_kernel_spmd(nc, in_maps, list(range(N_CORES)), **spmd_kwargs)
    out = np.concatenate(
        [res.results[i]["out"].astype(np.float32) for i in range(N_CORES)], axis=0
    )
    return out, res


def kernel(q, p, W):
    out, _ = run(q, p, W)
    return out


# revision 10
# speedup vs baseline: 1.0239x; 1.0067x over previous
"""Trainium2 Bass kernel for AttentionBilinear.

Per batch b:
    pW     = p[b] @ W                         # [Tp, Dq]
    scores = pW @ q[b].T                      # [Tp, Tq]
    wts    = softmax(scores, axis=Tp)
    out[b] = wts @ q[b]                       # [Tp, Dq]

Computed in the transposed-scores orientation so the softmax over Tp is a
free-axis reduction:
    pWT[d, tp]  = sum_e W[e, d] * pT[e, tp]       (mm1: lhsT=W,  rhs=pT)
    scT[tq, tp] = sum_d qT[d, tq] * pWT[d, tp]    (mm2: lhsT=qT, rhs=pWT)
    softmax over tp (free axis), read from PSUM   (DVE max / ACT exp / DVE mul)
    out[tp, d]  = sum_tq wT[tq, tp] * q[tq, d]    (mm3: lhsT=wT, rhs=q)

All matmuls run in fp16 (1 cycle/row on TRN2; fp8 DoubleRow measures the
same 215ns/instr for 2x the contraction, but single-term fp8 fails the
2e-2 gate at 3.5e-2, and multi-term splits erase the speed win — so fp16
is the PE floor: 768 matmuls x 216ns = 166us/core).

The schedule targets zero PE stalls after the framework prologue (~6us,
fixed): W is host-blocked k-major so one contiguous 256KB DMA delivers
the stationary tiles for a whole contraction chunk, batch 0's first pT
half arrives as 8 fine-grained 128KB pieces, and mm1(b=0, n=0) runs
k-outer across 8 PSUM banks so the PE starts on chunk 0 while chunk 1 is
still in flight. Output is staged to fp16 (host upcasts) and stored on
the scalar ring, which is idle after the W loads.

Sharding: data-parallel over batch B=16 across 8 cores, W replicated.
"""

import numpy as np

P = 128   # partitions
H = 512   # PSUM bank width in fp32

B_FULL = 16
T_FULL = 1024
D_FULL = 1024
N_CORES = 8

MODE = ("hi", "hi")  # kept for test.py's banner


def build_nc(b_loc=2, t=1024, d=1024):
    from contextlib import ExitStack

    import concourse.tile as tile
    from concourse import bacc, mybir

    f32 = mybir.dt.float32
    f16 = mybir.dt.float16
    C = t // P     # row chunks of a [t, d] matrix
    KC = d // P    # chunks of the d (feature) axis
    TH = t // H    # 512-wide pieces of the t axis
    NH = d // H    # 512-wide pieces of the d axis
    AX = mybir.AxisListType.X
    EXP = mybir.ActivationFunctionType.Exp
    MIN = mybir.AluOpType.min
    ADD = mybir.AluOpType.add

    nc = bacc.Bacc()

    # Inputs are host-packed partition-major ([b, p, c, cols]) so every DMA
    # descriptor is an 8-16KB contiguous run (vs 1-2KB row-major).
    def dram_in(name):
        return nc.dram_tensor(
            name, [b_loc, P, C, d], f16, kind="ExternalInput"
        ).ap()

    qh_ext = dram_in("qh")          # q natural, fp16
    qt_ext = dram_in("qt")          # q transposed per batch: [d, tq]
    # p transposed, additionally split into tp-halves: [b, TH, p, c, H]
    pt_ext = nc.dram_tensor(
        "pt", [b_loc, TH, P, C, H], f16, kind="ExternalInput"
    ).ap()
    # W host-blocked k-major as [k, p, m, c] = W[k*128+p, m*128+c]: the
    # piece for contraction chunk k is one contiguous 256KB DMA carrying
    # the stationary tiles for ALL output chunks m.
    w_ext = nc.dram_tensor("w", [KC, P, KC, P], f16, kind="ExternalInput").ap()
    # Output staged fp16 (host upcasts to fp32): halves the store traffic.
    out_ext = nc.dram_tensor("out", [b_loc, t, d], f16, kind="ExternalOutput").ap()

    with tile.TileContext(nc) as tc, ExitStack() as ctx:
        consts = ctx.enter_context(tc.tile_pool(name="consts", bufs=1))
        qh_pool = ctx.enter_context(tc.tile_pool(name="qh_pool", bufs=2))
        qt_pool = ctx.enter_context(tc.tile_pool(name="qt_pool", bufs=2))
        pt_pool = ctx.enter_context(tc.tile_pool(name="pt_pool", bufs=2))
        pwt_pool = ctx.enter_context(tc.tile_pool(name="pwt_pool", bufs=2))
        wt_pool = ctx.enter_context(tc.tile_pool(name="wt_pool", bufs=2))
        ostage = ctx.enter_context(tc.tile_pool(name="ostage", bufs=4))
        stats = ctx.enter_context(tc.tile_pool(name="stats", bufs=2))
        psum_mm = ctx.enter_context(tc.tile_pool(name="psum_mm", bufs=8, space="PSUM"))

        # ---- PE ramp warm-up: the clock takes ~3us of continuous busy to
        # reach 2.4GHz, and the first real data can't land before ~9.5us
        # (prologue + DMA pipeline). Nonzero junk matmuls burn the ramp
        # during the DMA wait so every real matmul runs at full clock.
        # (memset-zero operands get zero-skipped in ~40ns and don't ramp.)
        warm = consts.tile([P, H], f16, name="warm")
        nc.gpsimd.memset(warm[:], 1.0)
        wacc = psum_mm.tile([P, H], f32, name="wacc", tag="acc")
        for i in range(8):
            nc.tensor.matmul(
                wacc[:], warm[:, 0:P], warm[:], start=(i == 0), stop=(i == 7)
            )

        # ---- W resident (fp16), k-major blocks; lhsT for (k, m) is
        # w_[:, k, m, :]. One contiguous 256KB DMA per k, interleaved with
        # batch 0's pT pieces across BOTH rings in consumption order (even
        # chunks on sync, odd on scalar) so mm1's k-outer startup survives
        # the chip-wide HBM burst while all 8 cores load at once. ----
        w_ = consts.tile([P, KC, KC, P], f16, name="w_hi")

        st = [dict() for _ in range(b_loc)]

        def load_mat(pool, name, tag, ext, b, engine):
            """packed [b, P, C, d] DRAM (fp16) -> [P, C, d] SBUF in one DMA."""
            mt = pool.tile([P, C, d], f16, name=name, tag=tag)
            engine.dma_start(mt[:], ext[b])
            return mt

        def phase_loads(b):
            # All loads ride the sync ring as one FIFO in exact consumption
            # order, so early phases are never starved by later tensors.
            if b == 0:
                # Batch 0 startup: (W chunk k, pT half-0 chunk k) pairs in
                # exact k-outer consumption order, striped across both DMA
                # rings — each ring only needs ~110GB/s to keep the PE fed.
                pt0 = pt_pool.tile([P, C, d], f16, name="pT_0", tag="pT")
                for k in range(KC):
                    eng = nc.sync if k % 2 == 0 else nc.scalar
                    eng.dma_start(w_[:, k], w_ext[k])
                    eng.dma_start(pt0[:, k, 0:H], pt_ext[0, 0, :, k])
                for h in range(1, TH):
                    nc.sync.dma_start(
                        pt0[:, :, h * H : (h + 1) * H], pt_ext[0, h]
                    )
                st[0]["pT"] = pt0
            else:
                mt = pt_pool.tile([P, C, d], f16, name=f"pT_{b}", tag="pT")
                for h in range(TH):
                    nc.sync.dma_start(mt[:, :, h * H : (h + 1) * H], pt_ext[b, h])
                st[b]["pT"] = mt
            st[b]["qT"] = load_mat(qt_pool, f"qT_{b}", "qT", qt_ext, b, nc.sync)
            st[b]["qh"] = load_mat(qh_pool, f"qh_{b}", "qh", qh_ext, b, nc.sync)

        def phase_mm1(b):
            """pWT[d, tp] = sum_e W[e,d] * pT[e,tp]."""
            pT = st[b]["pT"]
            pWT = pwt_pool.tile([P, KC, t], f16, name=f"pWT_{b}", tag="pWT")
            if b == 0:
                # n=0 k-outer across 8 banks: each arriving (pT chunk k,
                # W chunk k) pair feeds 8 matmuls, so the PE is paced by
                # compute (1.7us/chunk) not DMA (0.45us/chunk) from the
                # first piece on.
                accs = [
                    psum_mm.tile([P, H], f32, name=f"a1_0_{m}_0", tag="acc")
                    for m in range(KC)
                ]
                for k in range(KC):
                    for m in range(KC):
                        nc.tensor.matmul(
                            accs[m][:],
                            w_[:, k, m, :],
                            pT[:, k, 0:H],
                            start=(k == 0),
                            stop=(k == KC - 1),
                        )
                for m in range(KC):
                    nc.scalar.copy(pWT[:, m, 0:H], accs[m][:])
                n_range = range(1, TH)
            else:
                n_range = range(TH)
            for n in n_range:
                n_sl = slice(n * H, (n + 1) * H)
                for m in range(KC):
                    acc = psum_mm.tile([P, H], f32, name=f"a1_{b}_{m}_{n}", tag="acc")
                    for k in range(KC):
                        nc.tensor.matmul(
                            acc[:],
                            w_[:, k, m, :],
                            pT[:, k, n_sl],
                            start=(k == 0),
                            stop=(k == KC - 1),
                        )
                    nc.scalar.copy(pWT[:, m, n_sl], acc[:])
            st[b]["pWT"] = pWT

        def phase_mm2sm(b):
            """scores into PSUM; softmax straight out of PSUM into fp16 wT."""
            qT = st[b]["qT"]
            pWT = st[b]["pWT"]
            wT = wt_pool.tile([P, C, t], f16, name=f"wT_{b}", tag="wT")
            negmax = stats.tile([P, C, TH], f32, name=f"negmax_{b}", tag="negmax")
            nm = stats.tile([P, C], f32, name=f"nm_{b}", tag="nm")
            sume = stats.tile([P, C, TH], f32, name=f"sume_{b}", tag="sume")
            recip = stats.tile([P, C], f32, name=f"recip_{b}", tag="recip")
            for m in range(C):
                msl = slice(m * P, (m + 1) * P)
                accs = []
                for n in range(TH):
                    n_sl = slice(n * H, (n + 1) * H)
                    acc = psum_mm.tile([P, H], f32, name=f"a2_{b}_{m}_{n}", tag="acc")
                    for k in range(KC):
                        nc.tensor.matmul(
                            acc[:],
                            qT[:, k, msl],
                            pWT[:, k, n_sl],
                            start=(k == 0),
                            stop=(k == KC - 1),
                        )
                    nc.vector.reduce_max(
                        negmax[:, m, n : n + 1], acc[:], axis=AX, negate=True
                    )
                    accs.append(acc)
                if TH > 1:
                    nc.vector.tensor_tensor(
                        nm[:, m : m + 1], negmax[:, m, 0:1], negmax[:, m, 1:2], op=MIN
                    )
                    nm_sl = nm[:, m : m + 1]
                else:
                    nm_sl = negmax[:, m, 0:1]
                for n, acc in enumerate(accs):
                    nc.scalar.activation(
                        wT[:, m, n * H : (n + 1) * H],
                        acc[:],
                        EXP,
                        bias=nm_sl,
                        accum_out=sume[:, m, n : n + 1],
                    )
                if TH > 1:
                    nc.vector.tensor_tensor(
                        recip[:, m : m + 1], sume[:, m, 0:1], sume[:, m, 1:2], op=ADD
                    )
                    nc.vector.reciprocal(recip[:, m : m + 1], recip[:, m : m + 1])
                else:
                    nc.vector.reciprocal(recip[:, m : m + 1], sume[:, m, 0:1])
                nc.vector.tensor_scalar_mul(wT[:, m, :], wT[:, m, :], recip[:, m : m + 1])
            st[b]["wT"] = wT

        def phase_mm3(b):
            """out[tp, d] = sum_tq wT[tq,tp] * qh[tq,d]."""
            wT = st[b]["wT"]
            qh = st[b]["qh"]
            for m in range(C):
                msl = slice(m * P, (m + 1) * P)
                for n in range(NH):
                    n_sl = slice(n * H, (n + 1) * H)
                    acc = psum_mm.tile([P, H], f32, name=f"a3_{b}_{m}_{n}", tag="acc")
                    for k in range(KC):
                        nc.tensor.matmul(
                            acc[:],
                            wT[:, k, msl],
                            qh[:, k, n_sl],
                            start=(k == 0),
                            stop=(k == KC - 1),
                        )
                    ot = ostage.tile([P, H], f16, name=f"ot_{b}_{m}_{n}", tag="ot")
                    nc.scalar.copy(ot[:], acc[:])
                    # Stores ride the scalar ring (idle after the W loads);
                    # the last batch alternates with the sync ring (idle
                    # after all loads) so the final drain runs at 2x.
                    if b == b_loc - 1 and (m * NH + n) % 2 == 0:
                        eng = nc.sync
                    else:
                        eng = nc.scalar
                    eng.dma_start(
                        out_ext[b, m * P : (m + 1) * P, n * H : (n + 1) * H], ot[:]
                    )

        # Emission order = per-engine program order. Batch b+1's mm1 is
        # emitted before batch b's mm3 so the PE stays busy while b's softmax
        # tail completes.
        phase_loads(0)
        phase_mm1(0)
        for b in range(b_loc):
            phase_mm2sm(b)
            if b + 1 < b_loc:
                phase_loads(b + 1)
                phase_mm1(b + 1)
            phase_mm3(b)

    nc.finalize()  # run the Bacc legalization/regalloc passes for walrus
    return nc


_CACHE = {}


def _get_nc():
    if "nc" not in _CACHE:
        _CACHE["nc"] = build_nc(B_FULL // N_CORES, T_FULL, D_FULL)
    return _CACHE["nc"]


def _prep_inputs(q, p, W):
    """Host-side layout prep: fp16 casts and per-batch transposes."""
    q = np.ascontiguousarray(q, dtype=np.float32)
    p = np.ascontiguousarray(p, dtype=np.float32)
    W = np.ascontiguousarray(W, dtype=np.float32)
    d = W.shape[0]
    KC = d // P
    t = q.shape[1]
    C = t // P
    TH = t // H

    def pack(x16):
        # [b, t, cols] -> [b, p, c, cols]: 16KB contiguous per partition
        b, _, cols = x16.shape
        return np.ascontiguousarray(
            x16.reshape(b, C, P, cols).transpose(0, 2, 1, 3)
        )

    def pack_halved(x16):
        # [b, t, cols] -> [b, h, p, c, H]: 8KB contiguous per partition
        b, _, cols = x16.shape
        return np.ascontiguousarray(
            x16.reshape(b, C, P, TH, H).transpose(0, 3, 2, 1, 4)
        )

    qh = q.astype(np.float16)
    qt = np.transpose(qh, (0, 2, 1))
    pt = np.transpose(p, (0, 2, 1)).astype(np.float16)
    # k-major W blocks: [k, p, m, c] = W[k*128+p, m*128+c] — a plain reshape
    wh = np.ascontiguousarray(W.astype(np.float16).reshape(KC, P, KC, P))
    return {
        "qh": pack(qh),
        "qt": pack(qt),
        "pt": pack_halved(pt),
        "w": wh,
    }


def run(q, p, W, nc=None, **spmd_kwargs):
    """Run on 8 NeuronCores; returns (out, BassKernelResults)."""
    from concourse.bass_utils import run_bass_kernel_spmd

    arrs = _prep_inputs(q, p, W)
    if nc is None:
        nc = _get_nc()
    bl = B_FULL // N_CORES
    batch_sharded = {"qh", "qt", "pt"}
    in_maps = []
    for i in range(N_CORES):
        m = {}
        for name, a in arrs.items():
            m[name] = a[i * bl : (i + 1) * bl] if name in batch_sharded else a
        in_maps.append(m)
    res = run_bass_kernel_spmd(nc, in_maps, list(range(N_CORES)), **spmd_kwargs)
    out = np.concatenate(
        [res.results[i]["out"].astype(np.float32) for i in range(N_CORES)], axis=0
    )
    return out, res


def kernel(q, p, W):
    out, _ = run(q, p, W)
    return out


# revision 12
# speedup vs baseline: 1.0247x; 1.0008x over previous
"""Trainium2 Bass kernel for AttentionBilinear.

Per batch b:
    pW     = p[b] @ W                         # [Tp, Dq]
    scores = pW @ q[b].T                      # [Tp, Tq]
    wts    = softmax(scores, axis=Tp)
    out[b] = wts @ q[b]                       # [Tp, Dq]

Computed in the transposed-scores orientation so the softmax over Tp is a
free-axis reduction:
    pWT[d, tp]  = sum_e W[e, d] * pT[e, tp]       (mm1: lhsT=W,  rhs=pT)
    scT[tq, tp] = sum_d qT[d, tq] * pWT[d, tp]    (mm2: lhsT=qT, rhs=pWT)
    softmax over tp (free axis), read from PSUM   (DVE max / ACT exp / DVE mul)
    out[tp, d]  = sum_tq wT[tq, tp] * q[tq, d]    (mm3: lhsT=wT, rhs=q)

All matmuls run in fp16 (1 cycle/row on TRN2; fp8 DoubleRow measures the
same 215ns/instr for 2x the contraction, but single-term fp8 fails the
2e-2 gate at 3.5e-2, and multi-term splits erase the speed win — so fp16
is the PE floor: 768 matmuls x 216ns = 166us/core).

The schedule targets zero PE stalls after the framework prologue (~6us,
fixed): W is host-blocked k-major so one contiguous 256KB DMA delivers
the stationary tiles for a whole contraction chunk, batch 0's first pT
half arrives as 8 fine-grained 128KB pieces, and mm1(b=0, n=0) runs
k-outer across 8 PSUM banks so the PE starts on chunk 0 while chunk 1 is
still in flight. Output is staged to fp16 (host upcasts) and stored on
the scalar ring, which is idle after the W loads.

Sharding: data-parallel over batch B=16 across 8 cores, W replicated.
"""

import numpy as np

P = 128   # partitions
H = 512   # PSUM bank width in fp32

B_FULL = 16
T_FULL = 1024
D_FULL = 1024
N_CORES = 8

MODE = ("hi", "hi")  # kept for test.py's banner


def build_nc(b_loc=2, t=1024, d=1024):
    from contextlib import ExitStack

    import concourse.tile as tile
    from concourse import bacc, mybir

    f32 = mybir.dt.float32
    f16 = mybir.dt.float16
    C = t // P     # row chunks of a [t, d] matrix
    KC = d // P    # chunks of the d (feature) axis
    TH = t // H    # 512-wide pieces of the t axis
    NH = d // H    # 512-wide pieces of the d axis
    AX = mybir.AxisListType.X
    EXP = mybir.ActivationFunctionType.Exp
    MIN = mybir.AluOpType.min
    ADD = mybir.AluOpType.add

    nc = bacc.Bacc()

    # Inputs are host-packed partition-major ([b, p, c, cols]) so every DMA
    # descriptor is an 8-16KB contiguous run (vs 1-2KB row-major).
    def dram_in(name):
        return nc.dram_tensor(
            name, [b_loc, P, C, d], f16, kind="ExternalInput"
        ).ap()

    qh_ext = dram_in("qh")          # q natural, fp16
    qt_ext = dram_in("qt")          # q transposed per batch: [d, tq]
    # p transposed, additionally split into tp-halves: [b, TH, p, c, H]
    pt_ext = nc.dram_tensor(
        "pt", [b_loc, TH, P, C, H], f16, kind="ExternalInput"
    ).ap()
    # W host-blocked k-major as [k, p, m, c] = W[k*128+p, m*128+c]: the
    # piece for contraction chunk k is one contiguous 256KB DMA carrying
    # the stationary tiles for ALL output chunks m.
    w_ext = nc.dram_tensor("w", [KC, P, KC, P], f16, kind="ExternalInput").ap()
    # Output staged fp16 (host upcasts to fp32): halves the store traffic.
    out_ext = nc.dram_tensor("out", [b_loc, t, d], f16, kind="ExternalOutput").ap()

    with tile.TileContext(nc) as tc, ExitStack() as ctx:
        consts = ctx.enter_context(tc.tile_pool(name="consts", bufs=1))
        qh_pool = ctx.enter_context(tc.tile_pool(name="qh_pool", bufs=2))
        qt_pool = ctx.enter_context(tc.tile_pool(name="qt_pool", bufs=2))
        pt_pool = ctx.enter_context(tc.tile_pool(name="pt_pool", bufs=2))
        pwt_pool = ctx.enter_context(tc.tile_pool(name="pwt_pool", bufs=2))
        wt_pool = ctx.enter_context(tc.tile_pool(name="wt_pool", bufs=2))
        ostage = ctx.enter_context(tc.tile_pool(name="ostage", bufs=4))
        stats = ctx.enter_context(tc.tile_pool(name="stats", bufs=2))
        psum_mm = ctx.enter_context(tc.tile_pool(name="psum_mm", bufs=8, space="PSUM"))

        # ---- PE ramp warm-up: the clock takes ~3us of continuous busy to
        # reach 2.4GHz, and the first real data can't land before ~9.5us
        # (prologue + DMA pipeline). Nonzero junk matmuls burn the ramp
        # during the DMA wait so every real matmul runs at full clock.
        # (memset-zero operands get zero-skipped in ~40ns and don't ramp.)
        warm = consts.tile([P, H], f16, name="warm")
        nc.gpsimd.memset(warm[:], 1.0)
        wacc = psum_mm.tile([P, H], f32, name="wacc", tag="acc")
        for i in range(10):
            nc.tensor.matmul(
                wacc[:], warm[:, 0:P], warm[:], start=(i == 0), stop=(i == 9)
            )

        # ---- W resident (fp16), k-major blocks; lhsT for (k, m) is
        # w_[:, k, m, :]. One contiguous 256KB DMA per k, interleaved with
        # batch 0's pT pieces across BOTH rings in consumption order (even
        # chunks on sync, odd on scalar) so mm1's k-outer startup survives
        # the chip-wide HBM burst while all 8 cores load at once. ----
        w_ = consts.tile([P, KC, KC, P], f16, name="w_hi")

        st = [dict() for _ in range(b_loc)]

        def load_mat(pool, name, tag, ext, b, engine):
            """packed [b, P, C, d] DRAM (fp16) -> [P, C, d] SBUF in one DMA."""
            mt = pool.tile([P, C, d], f16, name=name, tag=tag)
            engine.dma_start(mt[:], ext[b])
            return mt

        def phase_loads(b):
            # All loads ride the sync ring as one FIFO in exact consumption
            # order, so early phases are never starved by later tensors.
            if b == 0:
                # Batch 0 startup: (W chunk k, pT half-0 chunk k) pairs in
                # exact k-outer consumption order, striped across both DMA
                # rings — each ring only needs ~110GB/s to keep the PE fed.
                pt0 = pt_pool.tile([P, C, d], f16, name="pT_0", tag="pT")
                for k in range(KC):
                    eng = nc.sync if k % 2 == 0 else nc.scalar
                    eng.dma_start(w_[:, k], w_ext[k])
                    eng.dma_start(pt0[:, k, 0:H], pt_ext[0, 0, :, k])
                for h in range(1, TH):
                    nc.sync.dma_start(
                        pt0[:, :, h * H : (h + 1) * H], pt_ext[0, h]
                    )
                st[0]["pT"] = pt0
            else:
                mt = pt_pool.tile([P, C, d], f16, name=f"pT_{b}", tag="pT")
                for h in range(TH):
                    nc.sync.dma_start(mt[:, :, h * H : (h + 1) * H], pt_ext[b, h])
                st[b]["pT"] = mt
            st[b]["qT"] = load_mat(qt_pool, f"qT_{b}", "qT", qt_ext, b, nc.sync)
            st[b]["qh"] = load_mat(qh_pool, f"qh_{b}", "qh", qh_ext, b, nc.sync)

        def phase_mm1(b):
            """pWT[d, tp] = sum_e W[e,d] * pT[e,tp]."""
            pT = st[b]["pT"]
            pWT = pwt_pool.tile([P, KC, t], f16, name=f"pWT_{b}", tag="pWT")
            if b == 0:
                # n=0 k-outer across 8 banks: each arriving (pT chunk k,
                # W chunk k) pair feeds 8 matmuls, so the PE is paced by
                # compute (1.7us/chunk) not DMA (0.45us/chunk) from the
                # first piece on.
                accs = [
                    psum_mm.tile([P, H], f32, name=f"a1_0_{m}_0", tag="acc")
                    for m in range(KC)
                ]
                for k in range(KC):
                    for m in range(KC):
                        nc.tensor.matmul(
                            accs[m][:],
                            w_[:, k, m, :],
                            pT[:, k, 0:H],
                            start=(k == 0),
                            stop=(k == KC - 1),
                        )
                for m in range(KC):
                    nc.scalar.copy(pWT[:, m, 0:H], accs[m][:])
                n_range = range(1, TH)
            else:
                n_range = range(TH)
            for n in n_range:
                n_sl = slice(n * H, (n + 1) * H)
                for m in range(KC):
                    acc = psum_mm.tile([P, H], f32, name=f"a1_{b}_{m}_{n}", tag="acc")
                    for k in range(KC):
                        nc.tensor.matmul(
                            acc[:],
                            w_[:, k, m, :],
                            pT[:, k, n_sl],
                            start=(k == 0),
                            stop=(k == KC - 1),
                        )
                    nc.scalar.copy(pWT[:, m, n_sl], acc[:])
            st[b]["pWT"] = pWT

        def phase_mm2sm(b):
            """scores into PSUM; softmax straight out of PSUM into fp16 wT."""
            qT = st[b]["qT"]
            pWT = st[b]["pWT"]
            wT = wt_pool.tile([P, C, t], f16, name=f"wT_{b}", tag="wT")
            negmax = stats.tile([P, C, TH], f32, name=f"negmax_{b}", tag="negmax")
            nm = stats.tile([P, C], f32, name=f"nm_{b}", tag="nm")
            sume = stats.tile([P, C, TH], f32, name=f"sume_{b}", tag="sume")
            recip = stats.tile([P, C], f32, name=f"recip_{b}", tag="recip")
            for m in range(C):
                msl = slice(m * P, (m + 1) * P)
                accs = []
                for n in range(TH):
                    n_sl = slice(n * H, (n + 1) * H)
                    acc = psum_mm.tile([P, H], f32, name=f"a2_{b}_{m}_{n}", tag="acc")
                    for k in range(KC):
                        nc.tensor.matmul(
                            acc[:],
                            qT[:, k, msl],
                            pWT[:, k, n_sl],
                            start=(k == 0),
                            stop=(k == KC - 1),
                        )
                    nc.vector.reduce_max(
                        negmax[:, m, n : n + 1], acc[:], axis=AX, negate=True
                    )
                    accs.append(acc)
                if TH > 1:
                    nc.vector.tensor_tensor(
                        nm[:, m : m + 1], negmax[:, m, 0:1], negmax[:, m, 1:2], op=MIN
                    )
                    nm_sl = nm[:, m : m + 1]
                else:
                    nm_sl = negmax[:, m, 0:1]
                for n, acc in enumerate(accs):
                    nc.scalar.activation(
                        wT[:, m, n * H : (n + 1) * H],
                        acc[:],
                        EXP,
                        bias=nm_sl,
                        accum_out=sume[:, m, n : n + 1],
                    )
                if TH > 1:
                    nc.vector.tensor_tensor(
                        recip[:, m : m + 1], sume[:, m, 0:1], sume[:, m, 1:2], op=ADD
                    )
                    nc.vector.reciprocal(recip[:, m : m + 1], recip[:, m : m + 1])
                else:
                    nc.vector.reciprocal(recip[:, m : m + 1], sume[:, m, 0:1])
                nc.vector.tensor_scalar_mul(wT[:, m, :], wT[:, m, :], recip[:, m : m + 1])
            st[b]["wT"] = wT

        def phase_mm3(b):
            """out[tp, d] = sum_tq wT[tq,tp] * qh[tq,d]."""
            wT = st[b]["wT"]
            qh = st[b]["qh"]

            def store(m, n, ot):
                # Stores ride the scalar ring (idle after the W loads); the
                # last batch alternates with the sync ring (idle after all
                # loads), and the final tile splits across both by
                # partition halves, so the terminal drain runs at 2x.
                msl = slice(m * P, (m + 1) * P)
                n_sl = slice(n * H, (n + 1) * H)
                if b == b_loc - 1 and m == C - 1 and n == NH - 1:
                    hp = P // 2
                    nc.sync.dma_start(
                        out_ext[b, m * P : m * P + hp, n_sl], ot[0:hp]
                    )
                    nc.scalar.dma_start(
                        out_ext[b, m * P + hp : (m + 1) * P, n_sl], ot[hp:P]
                    )
                    return
                if b == b_loc - 1 and (m * NH + n) % 2 == 0:
                    eng = nc.sync
                else:
                    eng = nc.scalar
                eng.dma_start(out_ext[b, msl, n_sl], ot[:])

            # m=0: run k<KC-1 for both n-accs first, then the k=KC-1 terms —
            # gives the PE ~3us of work that doesn't need wT's last chunk,
            # hiding the final softmax chunk's DVE/ACT latency.
            msl0 = slice(0, P)
            accs0 = [
                psum_mm.tile([P, H], f32, name=f"a3_{b}_0_{n}", tag="acc")
                for n in range(NH)
            ]
            for k in range(KC - 1):
                for n in range(NH):
                    nc.tensor.matmul(
                        accs0[n][:],
                        wT[:, k, msl0],
                        qh[:, k, n * H : (n + 1) * H],
                        start=(k == 0),
                        stop=False,
                    )
            for n in range(NH):
                nc.tensor.matmul(
                    accs0[n][:],
                    wT[:, KC - 1, msl0],
                    qh[:, KC - 1, n * H : (n + 1) * H],
                    start=False,
                    stop=True,
                )
                ot = ostage.tile([P, H], f16, name=f"ot_{b}_0_{n}", tag="ot")
                nc.scalar.copy(ot[:], accs0[n][:])
                store(0, n, ot)
            for m in range(1, C):
                msl = slice(m * P, (m + 1) * P)
                for n in range(NH):
                    n_sl = slice(n * H, (n + 1) * H)
                    acc = psum_mm.tile([P, H], f32, name=f"a3_{b}_{m}_{n}", tag="acc")
                    for k in range(KC):
                        nc.tensor.matmul(
                            acc[:],
                            wT[:, k, msl],
                            qh[:, k, n_sl],
                            start=(k == 0),
                            stop=(k == KC - 1),
                        )
                    ot = ostage.tile([P, H], f16, name=f"ot_{b}_{m}_{n}", tag="ot")
                    nc.scalar.copy(ot[:], acc[:])
                    store(m, n, ot)

        # Emission order = per-engine program order. Batch b+1's mm1 is
        # emitted before batch b's mm3 so the PE stays busy while b's softmax
        # tail completes.
        phase_loads(0)
        phase_mm1(0)
        for b in range(b_loc):
            phase_mm2sm(b)
            if b + 1 < b_loc:
                phase_loads(b + 1)
                phase_mm1(b + 1)
            phase_mm3(b)

    nc.finalize()  # run the Bacc legalization/regalloc passes for walrus
    return nc


_CACHE = {}


def _get_nc():
    if "nc" not in _CACHE:
        _CACHE["nc"] = build_nc(B_FULL // N_CORES, T_FULL, D_FULL)
    return _CACHE["nc"]


def _prep_inputs(q, p, W):
    """Host-side layout prep: fp16 casts and per-batch transposes."""
    q = np.ascontiguousarray(q, dtype=np.float32)
    p = np.ascontiguousarray(p, dtype=np.float32)
    W = np.ascontiguousarray(W, dtype=np.float32)
    d = W.shape[0]
    KC = d // P
    t = q.shape[1]
    C = t // P
    TH = t // H

    def pack(x16):
        # [b, t, cols] -> [b, p, c, cols]: 16KB contiguous per partition
        b, _, cols = x16.shape
        return np.ascontiguousarray(
            x16.reshape(b, C, P, cols).transpose(0, 2, 1, 3)
        )

    def pack_halved(x16):
        # [b, t, cols] -> [b, h, p, c, H]: 8KB contiguous per partition
        b, _, cols = x16.shape
        return np.ascontiguousarray(
            x16.reshape(b, C, P, TH, H).transpose(0, 3, 2, 1, 4)
        )

    qh = q.astype(np.float16)
    qt = np.transpose(qh, (0, 2, 1))
    pt = np.transpose(p, (0, 2, 1)).astype(np.float16)
    # k-major W blocks: [k, p, m, c] = W[k*128+p, m*128+c] — a plain reshape
    wh = np.ascontiguousarray(W.astype(np.float16).reshape(KC, P, KC, P))
    return {
        "qh": pack(qh),
        "qt": pack(qt),
        "pt": pack_halved(pt),
        "w": wh,
    }


def run(q, p, W, nc=None, **spmd_kwargs):
    """Run on 8 NeuronCores; returns (out, BassKernelResults)."""
    from concourse.bass_utils import run_bass_kernel_spmd

    arrs = _prep_inputs(q, p, W)
    if nc is None:
        nc = _get_nc()
    bl = B_FULL // N_CORES
    batch_sharded = {"qh", "qt", "pt"}
    in_maps = []
    for i in range(N_CORES):
        m = {}
        for name, a in arrs.items():
            m[name] = a[i * bl : (i + 1) * bl] if name in batch_sharded else a
        in_maps.append(m)
    res = run_bass_kernel_spmd(nc, in_maps, list(range(N_CORES)), **spmd_kwargs)
    out = np.concatenate(
        [res.results[i]["out"].astype(np.float32) for i in range(N_CORES)], axis=0
    )
    return out, res


def kernel(q, p, W):
    out, _ = run(q, p, W)
    return out


# revision 13
# speedup vs baseline: 1.0259x; 1.0011x over previous
"""Trainium2 Bass kernel for AttentionBilinear.

Per batch b:
    pW     = p[b] @ W                         # [Tp, Dq]
    scores = pW @ q[b].T                      # [Tp, Tq]
    wts    = softmax(scores, axis=Tp)
    out[b] = wts @ q[b]                       # [Tp, Dq]

Computed in the transposed-scores orientation so the softmax over Tp is a
free-axis reduction:
    pWT[d, tp]  = sum_e W[e, d] * pT[e, tp]       (mm1: lhsT=W,  rhs=pT)
    scT[tq, tp] = sum_d qT[d, tq] * pWT[d, tp]    (mm2: lhsT=qT, rhs=pWT)
    softmax over tp (free axis), read from PSUM   (DVE max / ACT exp / DVE mul)
    out[tp, d]  = sum_tq wT[tq, tp] * q[tq, d]    (mm3: lhsT=wT, rhs=q)

All matmuls run in fp16 (1 cycle/row on TRN2; fp8 DoubleRow measures the
same 215ns/instr for 2x the contraction, but single-term fp8 fails the
2e-2 gate at 3.5e-2, and multi-term splits erase the speed win — so fp16
is the PE floor: 768 matmuls x 216ns = 166us/core).

The schedule targets zero PE stalls after the framework prologue (~6us,
fixed): W is host-blocked k-major so one contiguous 256KB DMA delivers
the stationary tiles for a whole contraction chunk, batch 0's first pT
half arrives as 8 fine-grained 128KB pieces, and mm1(b=0, n=0) runs
k-outer across 8 PSUM banks so the PE starts on chunk 0 while chunk 1 is
still in flight. Output is staged to fp16 (host upcasts) and stored on
the scalar ring, which is idle after the W loads.

Sharding: data-parallel over batch B=16 across 8 cores, W replicated.
"""

import numpy as np

P = 128   # partitions
H = 512   # PSUM bank width in fp32

B_FULL = 16
T_FULL = 1024
D_FULL = 1024
N_CORES = 8

MODE = ("hi", "hi")  # kept for test.py's banner


def build_nc(b_loc=2, t=1024, d=1024):
    from contextlib import ExitStack

    import concourse.tile as tile
    from concourse import bacc, mybir

    f32 = mybir.dt.float32
    f16 = mybir.dt.float16
    C = t // P     # row chunks of a [t, d] matrix
    KC = d // P    # chunks of the d (feature) axis
    TH = t // H    # 512-wide pieces of the t axis
    NH = d // H    # 512-wide pieces of the d axis
    AX = mybir.AxisListType.X
    EXP = mybir.ActivationFunctionType.Exp
    MIN = mybir.AluOpType.min
    ADD = mybir.AluOpType.add

    nc = bacc.Bacc()

    # Inputs are host-packed partition-major ([b, p, c, cols]) so every DMA
    # descriptor is an 8-16KB contiguous run (vs 1-2KB row-major).
    def dram_in(name):
        return nc.dram_tensor(
            name, [b_loc, P, C, d], f16, kind="ExternalInput"
        ).ap()

    qh_ext = dram_in("qh")          # q natural, fp16
    qt_ext = dram_in("qt")          # q transposed per batch: [d, tq]
    # p transposed, additionally split into tp-halves: [b, TH, p, c, H]
    pt_ext = nc.dram_tensor(
        "pt", [b_loc, TH, P, C, H], f16, kind="ExternalInput"
    ).ap()
    # W host-blocked k-major as [k, p, m, c] = W[k*128+p, m*128+c]: the
    # piece for contraction chunk k is one contiguous 256KB DMA carrying
    # the stationary tiles for ALL output chunks m.
    w_ext = nc.dram_tensor("w", [KC, P, KC, P], f16, kind="ExternalInput").ap()
    # Output staged fp16 (host upcasts to fp32): halves the store traffic.
    out_ext = nc.dram_tensor("out", [b_loc, t, d], f16, kind="ExternalOutput").ap()

    with tile.TileContext(nc) as tc, ExitStack() as ctx:
        consts = ctx.enter_context(tc.tile_pool(name="consts", bufs=1))
        qh_pool = ctx.enter_context(tc.tile_pool(name="qh_pool", bufs=2))
        qt_pool = ctx.enter_context(tc.tile_pool(name="qt_pool", bufs=2))
        pt_pool = ctx.enter_context(tc.tile_pool(name="pt_pool", bufs=2))
        pwt_pool = ctx.enter_context(tc.tile_pool(name="pwt_pool", bufs=2))
        wt_pool = ctx.enter_context(tc.tile_pool(name="wt_pool", bufs=2))
        ostage = ctx.enter_context(tc.tile_pool(name="ostage", bufs=4))
        stats = ctx.enter_context(tc.tile_pool(name="stats", bufs=2))
        psum_mm = ctx.enter_context(tc.tile_pool(name="psum_mm", bufs=8, space="PSUM"))

        # ---- PE ramp warm-up: the clock takes ~3us of continuous busy to
        # reach 2.4GHz, and the first real data can't land before ~9.5us
        # (prologue + DMA pipeline). Nonzero junk matmuls burn the ramp
        # during the DMA wait so every real matmul runs at full clock.
        # (memset-zero operands get zero-skipped in ~40ns and don't ramp.)
        warm = consts.tile([P, H], f16, name="warm")
        nc.gpsimd.memset(warm[:], 1.0)
        wacc = psum_mm.tile([P, H], f32, name="wacc", tag="acc")
        for i in range(10):
            nc.tensor.matmul(
                wacc[:], warm[:, 0:P], warm[:], start=(i == 0), stop=(i == 9)
            )

        # ---- W resident (fp16), k-major blocks; lhsT for (k, m) is
        # w_[:, k, m, :]. One contiguous 256KB DMA per k, interleaved with
        # batch 0's pT pieces across BOTH rings in consumption order (even
        # chunks on sync, odd on scalar) so mm1's k-outer startup survives
        # the chip-wide HBM burst while all 8 cores load at once. ----
        w_ = consts.tile([P, KC, KC, P], f16, name="w_hi")

        st = [dict() for _ in range(b_loc)]

        def load_mat(pool, name, tag, ext, b, engine):
            """packed [b, P, C, d] DRAM (fp16) -> [P, C, d] SBUF in one DMA."""
            mt = pool.tile([P, C, d], f16, name=name, tag=tag)
            engine.dma_start(mt[:], ext[b])
            return mt

        def phase_loads(b):
            # All loads ride the sync ring as one FIFO in exact consumption
            # order, so early phases are never starved by later tensors.
            if b == 0:
                # Batch 0 startup: (W chunk k, pT half-0 chunk k) pairs in
                # exact k-outer consumption order, striped across both DMA
                # rings — each ring only needs ~110GB/s to keep the PE fed.
                pt0 = pt_pool.tile([P, C, d], f16, name="pT_0", tag="pT")
                for k in range(KC):
                    eng = nc.sync if k % 2 == 0 else nc.scalar
                    eng.dma_start(w_[:, k], w_ext[k])
                    eng.dma_start(pt0[:, k, 0:H], pt_ext[0, 0, :, k])
                for h in range(1, TH):
                    nc.sync.dma_start(
                        pt0[:, :, h * H : (h + 1) * H], pt_ext[0, h]
                    )
                st[0]["pT"] = pt0
            else:
                mt = pt_pool.tile([P, C, d], f16, name=f"pT_{b}", tag="pT")
                for h in range(TH):
                    nc.sync.dma_start(mt[:, :, h * H : (h + 1) * H], pt_ext[b, h])
                st[b]["pT"] = mt
            st[b]["qT"] = load_mat(qt_pool, f"qT_{b}", "qT", qt_ext, b, nc.sync)
            st[b]["qh"] = load_mat(qh_pool, f"qh_{b}", "qh", qh_ext, b, nc.sync)

        def phase_mm1(b):
            """pWT[d, tp] = sum_e W[e,d] * pT[e,tp]."""
            pT = st[b]["pT"]
            pWT = pwt_pool.tile([P, KC, t], f16, name=f"pWT_{b}", tag="pWT")
            if b == 0:
                # n=0 k-outer across 8 banks: each arriving (pT chunk k,
                # W chunk k) pair feeds 8 matmuls, so the PE is paced by
                # compute (1.7us/chunk) not DMA (0.45us/chunk) from the
                # first piece on.
                accs = [
                    psum_mm.tile([P, H], f32, name=f"a1_0_{m}_0", tag="acc")
                    for m in range(KC)
                ]
                for k in range(KC):
                    for m in range(KC):
                        nc.tensor.matmul(
                            accs[m][:],
                            w_[:, k, m, :],
                            pT[:, k, 0:H],
                            start=(k == 0),
                            stop=(k == KC - 1),
                        )
                for m in range(KC):
                    nc.scalar.copy(pWT[:, m, 0:H], accs[m][:])
                n_range = range(1, TH)
            else:
                n_range = range(TH)
            for n in n_range:
                n_sl = slice(n * H, (n + 1) * H)
                for m in range(KC):
                    acc = psum_mm.tile([P, H], f32, name=f"a1_{b}_{m}_{n}", tag="acc")
                    for k in range(KC):
                        nc.tensor.matmul(
                            acc[:],
                            w_[:, k, m, :],
                            pT[:, k, n_sl],
                            start=(k == 0),
                            stop=(k == KC - 1),
                        )
                    nc.scalar.copy(pWT[:, m, n_sl], acc[:])
            st[b]["pWT"] = pWT

        def phase_mm2sm(b):
            """scores into PSUM; softmax straight out of PSUM into fp16 wT."""
            qT = st[b]["qT"]
            pWT = st[b]["pWT"]
            wT = wt_pool.tile([P, C, t], f16, name=f"wT_{b}", tag="wT")
            negmax = stats.tile([P, C, TH], f32, name=f"negmax_{b}", tag="negmax")
            nm = stats.tile([P, C], f32, name=f"nm_{b}", tag="nm")
            sume = stats.tile([P, C, TH], f32, name=f"sume_{b}", tag="sume")
            recip = stats.tile([P, C], f32, name=f"recip_{b}", tag="recip")
            for m in range(C):
                msl = slice(m * P, (m + 1) * P)
                accs = []
                for n in range(TH):
                    n_sl = slice(n * H, (n + 1) * H)
                    acc = psum_mm.tile([P, H], f32, name=f"a2_{b}_{m}_{n}", tag="acc")
                    for k in range(KC):
                        nc.tensor.matmul(
                            acc[:],
                            qT[:, k, msl],
                            pWT[:, k, n_sl],
                            start=(k == 0),
                            stop=(k == KC - 1),
                        )
                    nc.vector.reduce_max(
                        negmax[:, m, n : n + 1], acc[:], axis=AX, negate=True
                    )
                    accs.append(acc)
                if TH > 1:
                    nc.vector.tensor_tensor(
                        nm[:, m : m + 1], negmax[:, m, 0:1], negmax[:, m, 1:2], op=MIN
                    )
                    nm_sl = nm[:, m : m + 1]
                else:
                    nm_sl = negmax[:, m, 0:1]
                for n, acc in enumerate(accs):
                    nc.scalar.activation(
                        wT[:, m, n * H : (n + 1) * H],
                        acc[:],
                        EXP,
                        bias=nm_sl,
                        accum_out=sume[:, m, n : n + 1],
                    )
                if TH > 1:
                    nc.vector.tensor_tensor(
                        recip[:, m : m + 1], sume[:, m, 0:1], sume[:, m, 1:2], op=ADD
                    )
                    nc.vector.reciprocal(recip[:, m : m + 1], recip[:, m : m + 1])
                else:
                    nc.vector.reciprocal(recip[:, m : m + 1], sume[:, m, 0:1])
                nc.vector.tensor_scalar_mul(wT[:, m, :], wT[:, m, :], recip[:, m : m + 1])
            st[b]["wT"] = wT

        def phase_mm3(b):
            """out[tp, d] = sum_tq wT[tq,tp] * qh[tq,d]."""
            wT = st[b]["wT"]
            qh = st[b]["qh"]

            def store(m, n, ot):
                # Stores ride the scalar ring (idle after the W loads); the
                # last batch alternates with the sync ring (idle after all
                # loads), and the final tile splits across both by
                # partition halves, so the terminal drain runs at 2x.
                msl = slice(m * P, (m + 1) * P)
                n_sl = slice(n * H, (n + 1) * H)
                if b == b_loc - 1 and m == C - 1 and n == NH - 1:
                    hp = P // 2
                    nc.sync.dma_start(
                        out_ext[b, m * P : m * P + hp, n_sl], ot[0:hp]
                    )
                    nc.scalar.dma_start(
                        out_ext[b, m * P + hp : (m + 1) * P, n_sl], ot[hp:P]
                    )
                    return
                if b == b_loc - 1 and (m * NH + n) % 2 == 0:
                    eng = nc.sync
                else:
                    eng = nc.scalar
                eng.dma_start(out_ext[b, msl, n_sl], ot[:])

            # First two m-groups: run k<KC-1 for all four accs first, then
            # the k=KC-1 terms — gives the PE ~6us of work that doesn't
            # need wT's last chunk, hiding the final softmax chunk's
            # DVE/ACT latency (~4us).
            MI = 2
            accs0 = [
                psum_mm.tile([P, H], f32, name=f"a3_{b}_{mn // NH}_{mn % NH}", tag="acc")
                for mn in range(MI * NH)
            ]
            for k in range(KC - 1):
                for mn in range(MI * NH):
                    m, n = mn // NH, mn % NH
                    nc.tensor.matmul(
                        accs0[mn][:],
                        wT[:, k, m * P : (m + 1) * P],
                        qh[:, k, n * H : (n + 1) * H],
                        start=(k == 0),
                        stop=False,
                    )
            for mn in range(MI * NH):
                m, n = mn // NH, mn % NH
                nc.tensor.matmul(
                    accs0[mn][:],
                    wT[:, KC - 1, m * P : (m + 1) * P],
                    qh[:, KC - 1, n * H : (n + 1) * H],
                    start=False,
                    stop=True,
                )
                ot = ostage.tile([P, H], f16, name=f"ot_{b}_{m}_{n}", tag="ot")
                nc.scalar.copy(ot[:], accs0[mn][:])
                store(m, n, ot)
            for m in range(MI, C):
                msl = slice(m * P, (m + 1) * P)
                for n in range(NH):
                    n_sl = slice(n * H, (n + 1) * H)
                    acc = psum_mm.tile([P, H], f32, name=f"a3_{b}_{m}_{n}", tag="acc")
                    for k in range(KC):
                        nc.tensor.matmul(
                            acc[:],
                            wT[:, k, msl],
                            qh[:, k, n_sl],
                            start=(k == 0),
                            stop=(k == KC - 1),
                        )
                    ot = ostage.tile([P, H], f16, name=f"ot_{b}_{m}_{n}", tag="ot")
                    nc.scalar.copy(ot[:], acc[:])
                    store(m, n, ot)

        # Emission order = per-engine program order. Batch b+1's mm1 is
        # emitted before batch b's mm3 so the PE stays busy while b's softmax
        # tail completes.
        phase_loads(0)
        phase_mm1(0)
        for b in range(b_loc):
            phase_mm2sm(b)
            if b + 1 < b_loc:
                phase_loads(b + 1)
                phase_mm1(b + 1)
            phase_mm3(b)

    nc.finalize()  # run the Bacc legalization/regalloc passes for walrus
    return nc


_CACHE = {}


def _get_nc():
    if "nc" not in _CACHE:
        _CACHE["nc"] = build_nc(B_FULL // N_CORES, T_FULL, D_FULL)
    return _CACHE["nc"]


def _prep_inputs(q, p, W):
    """Host-side layout prep: fp16 casts and per-batch transposes."""
    q = np.ascontiguousarray(q, dtype=np.float32)
    p = np.ascontiguousarray(p, dtype=np.float32)
    W = np.ascontiguousarray(W, dtype=np.float32)
    d = W.shape[0]
    KC = d // P
    t = q.shape[1]
    C = t // P
    TH = t // H

    def pack(x16):
        # [b, t, cols] -> [b, p, c, cols]: 16KB contiguous per partition
        b, _, cols = x16.shape
        return np.ascontiguousarray(
            x16.reshape(b, C, P, cols).transpose(0, 2, 1, 3)
        )

    def pack_halved(x16):
        # [b, t, cols] -> [b, h, p, c, H]: 8KB contiguous per partition
        b, _, cols = x16.shape
        return np.ascontiguousarray(
            x16.reshape(b, C, P, TH, H).transpose(0, 3, 2, 1, 4)
        )

    qh = q.astype(np.float16)
    qt = np.transpose(qh, (0, 2, 1))
    pt = np.transpose(p, (0, 2, 1)).astype(np.float16)
    # k-major W blocks: [k, p, m, c] = W[k*128+p, m*128+c] — a plain reshape
    wh = np.ascontiguousarray(W.astype(np.float16).reshape(KC, P, KC, P))
    return {
        "qh": pack(qh),
        "qt": pack(qt),
        "pt": pack_halved(pt),
        "w": wh,
    }


def run(q, p, W, nc=None, **spmd_kwargs):
    """Run on 8 NeuronCores; returns (out, BassKernelResults)."""
    from concourse.bass_utils import run_bass_kernel_spmd

    arrs = _prep_inputs(q, p, W)
    if nc is None:
        nc = _get_nc()
    bl = B_FULL // N_CORES
    batch_sharded = {"qh", "qt", "pt"}
    in_maps = []
    for i in range(N_CORES):
        m = {}
        for name, a in arrs.items():
            m[name] = a[i * bl : (i + 1) * bl] if name in batch_sharded else a
        in_maps.append(m)
    res = run_bass_kernel_spmd(nc, in_maps, list(range(N_CORES)), **spmd_kwargs)
    out = np.concatenate(
        [res.results[i]["out"].astype(np.float32) for i in range(N_CORES)], axis=0
    )
    return out, res


def kernel(q, p, W):
    out, _ = run(q, p, W)
    return out
